# revision 30
# baseline (speedup 1.0000x reference)
"""Causal GQA self-attention with RoPE for TRN2, 8 NeuronCores.

Problem: B=2, S=2048, D=2048, H=16 q-heads, KV=4 kv-heads, HD=128.

Sharding: core c = (batch b = c//4, kv-group g = c%4). Each core computes
q-heads 4g..4g+3 and kv-head g for batch b; host sums the 4 partial
output projections per batch and transposes back.

Perf structure (PE-bound kernel, ~47ns fixed cost + 0.42ns/col per matmul):
  - K/V projections stream kd-outer so PE starts as soon as the first
    x chunk lands (x DMA overlaps projection compute).
  - Attention and output projection are fused jq-major: O-proj for query
    chunk jq runs right after its 4 heads finish, spreading the output
    DMA across the whole attention phase.
  - fp8(e4m3) DoubleRow matmuls (2x PE throughput) for PV, the softmax
    denominator (ones-matmul), and O-proj on jq>=1 (rows with >=512 keys,
    where fp8 noise averages out); jq=0 rows (few keys; these dominate
    the output max) stay fp16 end to end.
  - exp computed as exp(s*scale - 1) so fp8 probabilities can't overflow;
    the e^-1 factor cancels in the normalize.
  - Output partials in fp16 (halves the output DMA; host sums in fp32).
"""
import sys

sys.path.insert(0, "/opt/trn_rl_repo")

import numpy as np
import ml_dtypes

import concourse.tile as tile
from concourse import bacc, mybir
from concourse.bass_utils import run_bass_kernel_spmd

F32 = mybir.dt.float32
F16 = mybir.dt.float16
F8 = mybir.dt.float8e4
DR = mybir.MatmulPerfMode.DoubleRow
AF = mybir.ActivationFunctionType
OP = mybir.AluOpType

P = 128          # partitions / head dim
S = 2048         # sequence length
D = 2048         # model dim
NH = 4           # q heads per core
QW = NH * P      # q projection width per core (512)
NKD = D // P     # contraction chunks (16)
QCH = 512        # query chunk (free dim of attention matmuls)
NQC = S // QCH   # 4
KCH = P          # key chunk (128, on partitions)
NKC = S // KCH   # 16
SCALE = float(P) ** -0.5


def _host_constants():
    inv = 1.0 / (10000.0 ** (np.arange(0, P, 2, dtype=np.float64) / P))  # [64]
    pos = np.arange(S, dtype=np.float64)
    freqs = pos[:, None] * inv[None, :]                  # [S, 64]
    emb = np.concatenate([freqs, freqs], axis=-1)        # [S, 128]
    cosT = np.cos(emb).T.astype(np.float16).copy()       # [128, S]
    sinT = np.sin(emb).T.astype(np.float16)
    sinT[: P // 2] *= np.float16(-1.0)                   # fold rotate_half sign
    sinT = sinT.copy()
    # Causal masking via PE: tri[c, kp] = [c <= kp] (stationary) and
    # mbias[c, jd, q] = -30000*([q == c + 128*jd - 1] + [c == 0][q < 128*jd - 1])
    # (moving) so that (tri.T @ mbias[:, jd]) adds -30000 exactly on the
    # masked positions {q < kp + 128*jd} of a diagonal key block.
    c = np.arange(P)
    kp = np.arange(P)
    tri = (c[:, None] <= kp[None, :]).astype(np.float16)          # [c, kp]
    q = np.arange(QCH)
    mbias = np.zeros((P, 4, QCH), dtype=np.float16)
    for jd in range(4):
        hit = (q[None, :] == c[:, None] + 128 * jd - 1)
        full = (c[:, None] == 0) & (q[None, :] < 128 * jd - 1)
        mbias[:, jd, :] = np.float16(-30000.0) * (hit | full)
    ones16 = np.ones((P, P), dtype=np.float16)
    ones8 = np.ones((P, 2 * P), dtype=ml_dtypes.float8_e4m3fn)
    return cosT, sinT, tri, mbias, ones16, ones8


def build_nc():
    cosT_np, sinT_np, tri_np, mbias_np, ones16_np, ones8_np = _host_constants()

    nc = bacc.Bacc(None)
    xT_d = nc.dram_tensor("xT", [D, S], F16, kind="ExternalInput")
    wq_d = nc.dram_tensor("wq", [D, QW], F16, kind="ExternalInput")
    wk_d = nc.dram_tensor("wk", [D, P], F16, kind="ExternalInput")
    wv_d = nc.dram_tensor("wv", [D, P], F16, kind="ExternalInput")
    wo_d = nc.dram_tensor("wo", [QW, D], F16, kind="ExternalInput")
    out_d = nc.dram_tensor("outT", [D, S], F16, kind="ExternalOutput")

    cos_d = nc.inline_tensor(cosT_np, name="cosT")
    sin_d = nc.inline_tensor(sinT_np, name="sinT")
    tri_d = nc.inline_tensor(tri_np, name="tri")
    mbias_d = nc.inline_tensor(mbias_np, name="mbias")
    ones16_d = nc.inline_tensor(ones16_np, name="ones16")
    ones8_d = nc.inline_tensor(ones8_np, name="ones8")

    # DRAM views with the contraction dim split for SBUF partitions.
    xT_v = xT_d[:].rearrange("(kd p) s -> p kd s", p=P)
    wq_v = wq_d[:].rearrange("(kd p) c -> p kd c", p=P)
    wk_v = wk_d[:].rearrange("(kd p) c -> p kd c", p=P)
    wv_v = wv_d[:].rearrange("(kd p) c -> p kd c", p=P)
    wo_v = wo_d[:].rearrange("(a p) o -> p a o", p=P)

    with tile.TileContext(nc) as tc:
        with tc.tile_pool(name="persist", bufs=1) as pp, \
             tc.tile_pool(name="pswap", bufs=2) as pswap, \
             tc.tile_pool(name="ppt", bufs=4) as ppt, \
             tc.tile_pool(name="pdib", bufs=2) as pdib, \
             tc.tile_pool(name="p3", bufs=4) as p3, \
             tc.tile_pool(name="ps", bufs=1, space="PSUM") as psp:
            qT = pp.tile([P, NH, S], F16)
            vT = pp.tile([P, S], F16)
            attnT16 = pp.tile([P, NH, QCH], F16)
            kT = pp.tile([P, S], F16)
            vKf = pp.tile([P, NKC, P], F16)      # V as (kpos, kchunk, hd)
            vK8 = pp.tile([P, NKC, P], F8)
            attnT8 = pp.tile([P, NH, S], F8)      # jq>=1 attention out
            wo16 = pp.tile([P, NH, D], F16)
            wo8 = pp.tile([P, NH, D], F8)
            tri_t = pp.tile([P, P], F16)
            mbias_t = pp.tile([P, 4, QCH], F16)
            ones16_t = pp.tile([P, P], F16)
            ones8_t = pp.tile([P, 2, P], F8)
            cos_t = pp.tile([P, S], F16)
            sin_t = pp.tile([P, S], F16)
            eb = pp.tile([P, 1], F32)
            nc.gpsimd.memset(eb[:], -1.0)

            wkt = pp.tile([P, NKD, P], F16)
            wvt = pp.tile([P, NKD, P], F16)
            wqt = pp.tile([P, NKD, QW], F16)
            xf = pp.tile([P, NKD, S], F16)

            # DMA priority order: first-needed first.
            nc.sync.dma_start(tri_t[:], tri_d[:])
            nc.sync.dma_start(wkt[:], wk_v)
            for jr in range(NQC):
                nc.sync.dma_start(
                    xf[:, 0, jr * QCH : (jr + 1) * QCH],
                    xT_v[:, 0, jr * QCH : (jr + 1) * QCH],
                )
            nc.sync.dma_start(wvt[:], wv_v)
            for kd in range(1, NKD):
                nc.sync.dma_start(xf[:, kd, :], xT_v[:, kd, :])
            nc.sync.dma_start(cos_t[:], cos_d[:])
            nc.sync.dma_start(sin_t[:], sin_d[:])
            nc.sync.dma_start(mbias_t[:], mbias_d[:])
            nc.sync.dma_start(ones16_t[:], ones16_d[:])
            nc.sync.dma_start(
                ones8_t[:], ones8_d[:].rearrange("p (a b) -> p a b", a=2)
            )
            nc.sync.dma_start(wqt[:], wq_v)
            nc.sync.dma_start(wo16[:], wo_v)
            nc.scalar.copy(out=wo8[:], in_=wo16[:])

            # PSUM tags: A,B = [128,1024] (2 banks each); C..F = [128,512].
            def psA(name):
                return psp.tile([P, 2 * QCH], F32, tag=name, name=name)

            def psB(name):
                return psp.tile([P, QCH], F32, tag=name, name=name)

            def rope(dst_ap):
                sw = pswap.tile([P, S], F16, tag="swap")
                half = P // 2
                nc.sync.dma_start(sw[:half, :], dst_ap[half:, :])
                nc.sync.dma_start(sw[half:, :], dst_ap[:half, :])
                nc.vector.tensor_tensor(sw[:], sw[:], sin_t[:], OP.mult)
                nc.vector.tensor_tensor(dst_ap, dst_ap, cos_t[:], OP.mult)
                nc.vector.tensor_tensor(dst_ap, dst_ap, sw[:], OP.add)

            # PE warmup while the first DMAs stream in: eb is memset on
            # device (no DMA dependency), so the PE can start ramping within
            # ~2us of kernel start; a few tri_t matmuls then keep it busy
            # until the first x chunk lands.
            wps = psB("C")
            for i in range(110):
                nc.tensor.matmul(
                    wps[0:1, 0:1], eb[:], eb[:], start=True, stop=True
                )
            for i in range(16):
                nc.tensor.matmul(
                    wps[:, 0:P], tri_t[:], tri_t[:], start=True, stop=True
                )

            # ======== Phase 1: QKV projections ==========================
            # K/V stream kd-outer (start on the first x chunk); K uses
            # banks A,B (1024-wide halves), V uses C..F.
            kA, kB = psA("A"), psA("B")
            vps = [psB(t) for t in "CDEF"]
            for kd in range(NKD):
                for jr in range(NQC):
                    dst = (kA, kB)[jr // 2][:, (jr % 2) * QCH : (jr % 2 + 1) * QCH]
                    nc.tensor.matmul(
                        dst,
                        wkt[:, kd, :],
                        xf[:, kd, jr * QCH : (jr + 1) * QCH],
                        start=(kd == 0),
                        stop=(kd == NKD - 1),
                    )
                for jr in range(NQC):
                    nc.tensor.matmul(
                        vps[jr][:],
                        wvt[:, kd, :],
                        xf[:, kd, jr * QCH : (jr + 1) * QCH],
                        start=(kd == 0),
                        stop=(kd == NKD - 1),
                    )
            nc.scalar.copy(out=kT[:, 0 : 2 * QCH], in_=kA[:])
            nc.scalar.copy(out=kT[:, 2 * QCH : S], in_=kB[:])
            rope(kT[:])
            for jr in range(NQC):
                nc.scalar.copy(
                    out=vT[:, jr * QCH : (jr + 1) * QCH], in_=vps[jr][:]
                )
            nc.sync.dma_start_transpose(vKf[:], vT[:])
            nc.scalar.copy(out=vK8[:], in_=vKf[:])

            # Q heads: even heads use banks A,B; odd heads use C..F so the
            # next head's matmuls never WAR-stall on the previous head's
            # PSUM->SBUF copies. The first jq0 attention group is emitted
            # between q1 and q2 so its exps overlap the q2/q3 projections.
            def proj_q(hh):
                if hh % 2 == 0:
                    qab = [psA("A"), psA("B")]
                    qdst = [
                        qab[jr // 2][:, (jr % 2) * QCH : (jr % 2 + 1) * QCH]
                        for jr in range(NQC)
                    ]
                else:
                    qcf = [psB(t) for t in "CDEF"]
                    qdst = [qcf[jr][:] for jr in range(NQC)]
                for kd in range(NKD):
                    for jr in range(NQC):
                        nc.tensor.matmul(
                            qdst[jr],
                            wqt[:, kd, hh * P : (hh + 1) * P],
                            xf[:, kd, jr * QCH : (jr + 1) * QCH],
                            start=(kd == 0),
                            stop=(kd == NKD - 1),
                        )
                if hh % 2 == 0:
                    nc.scalar.copy(out=qT[:, hh, 0 : 2 * QCH], in_=qab[0][:])
                    nc.scalar.copy(out=qT[:, hh, 2 * QCH : S], in_=qab[1][:])
                else:
                    for jr in range(NQC):
                        nc.scalar.copy(
                            out=qT[:, hh, jr * QCH : (jr + 1) * QCH],
                            in_=qcf[jr][:],
                        )
                rope(qT[:, hh, :])

            # ======== Fused attention + output projection ================
            # Two heads co-scheduled per group; sps double-buffers on the
            # global pair index so QK(next) overlaps exp(cur). The second
            # head walks its key pairs in reverse so the two heads' diagonal
            # (masked) pairs never land in the same slot (spreads DVE work).
            pair_it = [0]
            pending_units = []

            def next_ab():
                sps = psA("A" if pair_it[0] % 2 == 0 else "B")
                pair_it[0] += 1
                return sps

            def attn_group(ha, hb, jq):
                fp8 = jq >= 1
                nkc = 4 * (jq + 1)
                npair = nkc // 2
                ops = {ha: psB("C"), hb: psB("D")}
                dps = {ha: psB("E"), hb: psB("F")}
                qs = {
                    h: qT[:, h, jq * QCH : (jq + 1) * QCH] for h in (ha, hb)
                }
                seq = []
                for i in range(npair):
                    seq.append((ha, i))
                    seq.append((hb, npair - 1 - i))
                first = {ha: 0, hb: 1}
                last = {ha: len(seq) - 2, hb: len(seq) - 1}

                def emit_qk(h, ip):
                    sps = next_ab()
                    kc0 = 2 * ip
                    for k2 in range(2):
                        jd = kc0 + k2 - 4 * jq
                        diag = jd >= 0
                        half = sps[:, k2 * QCH : (k2 + 1) * QCH]
                        nc.tensor.matmul(
                            half,
                            kT[:, (kc0 + k2) * P : (kc0 + k2 + 1) * P],
                            qs[h],
                            start=True,
                            stop=not diag,
                        )
                        if diag:
                            ncols = min(QCH, P * (jd + 1))
                            nc.tensor.matmul(
                                half[:, :ncols],
                                tri_t[:],
                                mbias_t[:, jd, :ncols],
                                start=False,
                                stop=True,
                            )
                    return sps

                sps_cur = emit_qk(*seq[0])
                for i, (h, ip) in enumerate(seq):
                    kc0 = 2 * ip
                    if fp8:
                        pT = ppt.tile([P, 2, QCH], F8, tag="pT8")
                    else:
                        pT = ppt.tile([P, 2, QCH], F16, tag="pT16")
                    pflat = pT[:].rearrange("p a b -> p (a b)")
                    nc.scalar.activation(
                        pflat, sps_cur[:], AF.Exp, scale=SCALE, bias=eb[:]
                    )
                    if i + 1 < len(seq):
                        sps_cur = emit_qk(*seq[i + 1])
                    if fp8:
                        nc.tensor.matmul(
                            ops[h][:],
                            vK8[:, kc0 : kc0 + 2, :],
                            pT[:],
                            start=(i == first[h]),
                            stop=(i == last[h]),
                            perf_mode=DR,
                        )
                        nc.tensor.matmul(
                            dps[h][:],
                            ones8_t[:],
                            pT[:],
                            start=(i == first[h]),
                            stop=(i == last[h]),
                            perf_mode=DR,
                        )
                    else:
                        for k2 in range(2):
                            nc.tensor.matmul(
                                ops[h][:],
                                vKf[:, kc0 + k2, :],
                                pT[:, k2, :],
                                start=(i == first[h] and k2 == 0),
                                stop=(i == last[h] and k2 == 1),
                            )
                            nc.tensor.matmul(
                                dps[h][:],
                                ones16_t[:],
                                pT[:, k2, :],
                                start=(i == first[h] and k2 == 0),
                                stop=(i == last[h] and k2 == 1),
                            )
                    if i % 2 == 1 and pending_units:
                        pending_units.pop(0)()
                for h in (ha, hb):
                    dib = pdib.tile([P, QCH], F32, tag="dib")
                    nc.vector.reciprocal_approx_fast(dib[:], dps[h][:])
                    if fp8:
                        dst = attnT8[:, h, jq * QCH : (jq + 1) * QCH]
                    else:
                        dst = attnT16[:, h, :]
                    nc.vector.tensor_tensor(dst, ops[h][:], dib[:], OP.mult)

            def make_oproj_units(jq):
                units = []
                for op_i in range(D // P // 2):
                    def unit(op_i=op_i, jq=jq):
                        ps = next_ab()
                        for i2 in range(2):
                            oc = 2 * op_i + i2
                            half = ps[:, i2 * QCH : (i2 + 1) * QCH]
                            if jq == 0:
                                for a in range(NH):
                                    nc.tensor.matmul(
                                        half,
                                        wo16[:, a, oc * P : (oc + 1) * P],
                                        attnT16[:, a, :],
                                        start=(a == 0),
                                        stop=(a == NH - 1),
                                    )
                            else:
                                for a2 in range(0, NH, 2):
                                    nc.tensor.matmul(
                                        half,
                                        wo8[:, a2 : a2 + 2, oc * P : (oc + 1) * P],
                                        attnT8[
                                            :,
                                            a2 : a2 + 2,
                                            jq * QCH : (jq + 1) * QCH,
                                        ],
                                        start=(a2 == 0),
                                        stop=(a2 == NH - 2),
                                        perf_mode=DR,
                                    )
                        ot = p3.tile([P, 2 * QCH], F16, tag="ot")
                        tail = jq == NQC - 1 and op_i >= 6
                        if tail:
                            # last units: split halves across both engines so
                            # the copy+DMA tail drains in parallel
                            nc.scalar.copy(out=ot[:, :QCH], in_=ps[:, :QCH])
                            nc.vector.tensor_copy(
                                out=ot[:, QCH:], in_=ps[:, QCH:]
                            )
                        elif op_i % 2 == 0:
                            nc.scalar.copy(out=ot[:], in_=ps[:])
                        else:
                            nc.vector.tensor_copy(out=ot[:], in_=ps[:])
                        for i2 in range(2):
                            oc = 2 * op_i + i2
                            nc.sync.dma_start(
                                out_d[
                                    oc * P : (oc + 1) * P,
                                    jq * QCH : (jq + 1) * QCH,
                                ],
                                ot[:, i2 * QCH : (i2 + 1) * QCH],
                            )
                    units.append(unit)
                return units

            proj_q(0)
            proj_q(1)
            proj_q(2)
            attn_group(0, 1, 0)
            proj_q(3)
            attn_group(2, 3, 0)
            pending_units.extend(make_oproj_units(0))
            for jq in range(1, NQC):
                attn_group(0, 1, jq)
                attn_group(2, 3, jq)
                pending_units.extend(make_oproj_units(jq))
            while pending_units:
                pending_units.pop(0)()

    nc.finalize()
    return nc


_NC = None


def _get_nc():
    global _NC
    if _NC is None:
        _NC = build_nc()
    return _NC


def make_in_maps(x, wq, wk, wv, wo):
    x = np.asarray(x, dtype=np.float32)
    f16 = np.float16
    in_maps = []
    for c in range(8):
        b, g = c // 4, c % 4
        in_maps.append(
            {
                "xT": np.ascontiguousarray(x[b].T).astype(f16),
                "wq": np.asarray(wq[:, QW * g : QW * (g + 1)], dtype=f16),
                "wk": np.asarray(wk[:, P * g : P * (g + 1)], dtype=f16),
                "wv": np.asarray(wv[:, P * g : P * (g + 1)], dtype=f16),
                "wo": np.asarray(wo[QW * g : QW * (g + 1), :], dtype=f16),
            }
        )
    return in_maps


def kernel(x, wq, wk, wv, wo):
    nc = _get_nc()
    in_maps = make_in_maps(x, wq, wk, wv, wo)
    res = run_bass_kernel_spmd(nc, in_maps, list(range(8)))
    parts = [res.results[c]["outT"].astype(np.float32) for c in range(8)]
    out = np.stack(
        [
            (parts[0] + parts[1] + parts[2] + parts[3]).T,
            (parts[4] + parts[5] + parts[6] + parts[7]).T,
        ]
    ).astype(np.float32)
    return out


# revision 31
# speedup vs baseline: 1.1902x; 1.1902x over previous
"""Causal GQA self-attention with RoPE for TRN2, 8 NeuronCores.

Problem: B=2, S=2048, D=2048, H=16 q-heads, KV=4 kv-heads, HD=128.

Sharding: core c = (batch b = c//4, kv-group g = c%4). Each core computes
q-heads 4g..4g+3 and kv-head g for batch b; host sums the 4 partial
output projections per batch and transposes back.

Perf structure (PE-bound kernel, ~47ns fixed cost + 0.42ns/col per matmul):
  - K/V projections stream kd-outer so PE starts as soon as the first
    x chunk lands (x DMA overlaps projection compute).
  - Attention and output projection are fused jq-major: O-proj for query
    chunk jq runs right after its 4 heads finish, spreading the output
    DMA across the whole attention phase.
  - fp8(e4m3) DoubleRow matmuls (2x PE throughput) for PV, the softmax
    denominator (ones-matmul), and O-proj on jq>=1 (rows with >=512 keys,
    where fp8 noise averages out); jq=0 rows (few keys; these dominate
    the output max) stay fp16 end to end.
  - exp computed as exp(s*scale - 1) so fp8 probabilities can't overflow;
    the e^-1 factor cancels in the normalize.
  - Output partials in fp16 (halves the output DMA; host sums in fp32).
"""
import sys

sys.path.insert(0, "/opt/trn_rl_repo")

import numpy as np
import ml_dtypes

import concourse.tile as tile
from concourse import bacc, mybir
from concourse.bass_utils import run_bass_kernel_spmd

F32 = mybir.dt.float32
F16 = mybir.dt.float16
F8 = mybir.dt.float8e4
DR = mybir.MatmulPerfMode.DoubleRow
AF = mybir.ActivationFunctionType
OP = mybir.AluOpType

P = 128          # partitions / head dim
S = 2048         # sequence length
D = 2048         # model dim
NH = 4           # q heads per core
QW = NH * P      # q projection width per core (512)
NKD = D // P     # contraction chunks (16)
QCH = 512        # query chunk (free dim of attention matmuls)
NQC = S // QCH   # 4
KCH = P          # key chunk (128, on partitions)
NKC = S // KCH   # 16
SCALE = float(P) ** -0.5


def _host_constants():
    inv = 1.0 / (10000.0 ** (np.arange(0, P, 2, dtype=np.float64) / P))  # [64]
    pos = np.arange(S, dtype=np.float64)
    freqs = pos[:, None] * inv[None, :]                  # [S, 64]
    emb = np.concatenate([freqs, freqs], axis=-1)        # [S, 128]
    cosT = np.cos(emb).T.astype(np.float16).copy()       # [128, S]
    sinT = np.sin(emb).T.astype(np.float16)
    sinT[: P // 2] *= np.float16(-1.0)                   # fold rotate_half sign
    sinT = sinT.copy()
    # Causal masking via PE: tri[c, kp] = [c <= kp] (stationary) and
    # mbias[c, jd, q] = -30000*([q == c + 128*jd - 1] + [c == 0][q < 128*jd - 1])
    # (moving) so that (tri.T @ mbias[:, jd]) adds -30000 exactly on the
    # masked positions {q < kp + 128*jd} of a diagonal key block.
    c = np.arange(P)
    kp = np.arange(P)
    tri = (c[:, None] <= kp[None, :]).astype(np.float16)          # [c, kp]
    q = np.arange(QCH)
    mbias = np.zeros((P, 4, QCH), dtype=np.float16)
    for jd in range(4):
        hit = (q[None, :] == c[:, None] + 128 * jd - 1)
        full = (c[:, None] == 0) & (q[None, :] < 128 * jd - 1)
        mbias[:, jd, :] = np.float16(-30000.0) * (hit | full)
    ones16 = np.ones((P, P), dtype=np.float16)
    ones8 = np.ones((P, 2 * P), dtype=ml_dtypes.float8_e4m3fn)
    return cosT, sinT, tri, mbias, ones16, ones8


def build_nc():
    cosT_np, sinT_np, tri_np, mbias_np, ones16_np, ones8_np = _host_constants()

    nc = bacc.Bacc(None)
    xT_d = nc.dram_tensor("xT", [D, S], F16, kind="ExternalInput")
    wq_d = nc.dram_tensor("wq", [D, QW], F16, kind="ExternalInput")
    wk_d = nc.dram_tensor("wk", [D, P], F16, kind="ExternalInput")
    wv_d = nc.dram_tensor("wv", [D, P], F16, kind="ExternalInput")
    wo_d = nc.dram_tensor("wo", [QW, D], F16, kind="ExternalInput")
    out_d = nc.dram_tensor("outT", [D, S], F16, kind="ExternalOutput")

    cos_d = nc.inline_tensor(cosT_np, name="cosT")
    sin_d = nc.inline_tensor(sinT_np, name="sinT")
    tri_d = nc.inline_tensor(tri_np, name="tri")
    mbias_d = nc.inline_tensor(mbias_np, name="mbias")
    ones16_d = nc.inline_tensor(ones16_np, name="ones16")
    ones8_d = nc.inline_tensor(ones8_np, name="ones8")

    # DRAM views with the contraction dim split for SBUF partitions.
    xT_v = xT_d[:].rearrange("(kd p) s -> p kd s", p=P)
    wq_v = wq_d[:].rearrange("(kd p) c -> p kd c", p=P)
    wk_v = wk_d[:].rearrange("(kd p) c -> p kd c", p=P)
    wv_v = wv_d[:].rearrange("(kd p) c -> p kd c", p=P)
    wo_v = wo_d[:].rearrange("(a p) o -> p a o", p=P)

    with tile.TileContext(nc) as tc:
        with tc.tile_pool(name="persist", bufs=1) as pp, \
             tc.tile_pool(name="pswap", bufs=2) as pswap, \
             tc.tile_pool(name="ppt", bufs=4) as ppt, \
             tc.tile_pool(name="pdib", bufs=2) as pdib, \
             tc.tile_pool(name="p3", bufs=4) as p3, \
             tc.tile_pool(name="ps", bufs=1, space="PSUM") as psp:
            qT = pp.tile([P, NH, S], F16)
            vT = pp.tile([P, S], F16)
            attnT16 = pp.tile([P, NH, QCH], F16)
            kT = pp.tile([P, S], F16)
            vKf = pp.tile([P, NKC, P], F16)      # V as (kpos, kchunk, hd)
            vK8 = pp.tile([P, NKC, P], F8)
            attnT8 = pp.tile([P, NH, S], F8)      # jq>=1 attention out
            wo16 = pp.tile([P, NH, D], F16)
            wo8 = pp.tile([P, NH, D], F8)
            tri_t = pp.tile([P, P], F16)
            mbias_t = pp.tile([P, 4, QCH], F16)
            ones16_t = pp.tile([P, P], F16)
            ones8_t = pp.tile([P, 2, P], F8)
            cos_t = pp.tile([P, S], F16)
            sin_t = pp.tile([P, S], F16)
            eb = pp.tile([P, 1], F32)
            nc.gpsimd.memset(eb[:], -1.0)

            wkt = pp.tile([P, NKD, P], F16)
            wvt = pp.tile([P, NKD, P], F16)
            wqt = pp.tile([P, NKD, QW], F16)
            xf = pp.tile([P, NKD, S], F16)

            # DMA priority order: first-needed first.
            nc.sync.dma_start(tri_t[:], tri_d[:])
            nc.sync.dma_start(wkt[:], wk_v)
            for jr in range(NQC):
                nc.sync.dma_start(
                    xf[:, 0, jr * QCH : (jr + 1) * QCH],
                    xT_v[:, 0, jr * QCH : (jr + 1) * QCH],
                )
            nc.sync.dma_start(wvt[:], wv_v)
            for kd in range(1, NKD):
                nc.sync.dma_start(xf[:, kd, :], xT_v[:, kd, :])
            nc.sync.dma_start(cos_t[:], cos_d[:])
            nc.sync.dma_start(sin_t[:], sin_d[:])
            nc.sync.dma_start(mbias_t[:], mbias_d[:])
            nc.sync.dma_start(ones16_t[:], ones16_d[:])
            nc.sync.dma_start(
                ones8_t[:], ones8_d[:].rearrange("p (a b) -> p a b", a=2)
            )
            nc.sync.dma_start(wqt[:], wq_v)
            nc.sync.dma_start(wo16[:], wo_v)
            nc.scalar.copy(out=wo8[:], in_=wo16[:])

            # PSUM tags: A,B = [128,1024] (2 banks each); C..F = [128,512].
            def psA(name):
                return psp.tile([P, 2 * QCH], F32, tag=name, name=name)

            def psB(name):
                return psp.tile([P, QCH], F32, tag=name, name=name)

            def rope(dst_ap):
                sw = pswap.tile([P, S], F16, tag="swap")
                half = P // 2
                nc.sync.dma_start(sw[:half, :], dst_ap[half:, :])
                nc.sync.dma_start(sw[half:, :], dst_ap[:half, :])
                nc.vector.tensor_tensor(sw[:], sw[:], sin_t[:], OP.mult)
                nc.vector.tensor_tensor(dst_ap, dst_ap, cos_t[:], OP.mult)
                nc.vector.tensor_tensor(dst_ap, dst_ap, sw[:], OP.add)

            # PE warmup while the first DMAs stream in: eb is memset on
            # device (no DMA dependency), so the PE can start ramping within
            # ~2us of kernel start; a few tri_t matmuls then keep it busy
            # until the first x chunk lands.
            wps = psB("C")
            for i in range(110):
                nc.tensor.matmul(
                    wps[0:1, 0:1], eb[:], eb[:], start=True, stop=True
                )
            for i in range(16):
                nc.tensor.matmul(
                    wps[:, 0:P], tri_t[:], tri_t[:], start=True, stop=True
                )

            # ======== Phase 1: QKV projections ==========================
            # K/V stream kd-outer (start on the first x chunk); K uses
            # banks A,B (1024-wide halves), V uses C..F.
            kA, kB = psA("A"), psA("B")
            vps = [psB(t) for t in "CDEF"]
            for kd in range(NKD):
                for jr in range(NQC):
                    dst = (kA, kB)[jr // 2][:, (jr % 2) * QCH : (jr % 2 + 1) * QCH]
                    nc.tensor.matmul(
                        dst,
                        wkt[:, kd, :],
                        xf[:, kd, jr * QCH : (jr + 1) * QCH],
                        start=(kd == 0),
                        stop=(kd == NKD - 1),
                    )
                for jr in range(NQC):
                    nc.tensor.matmul(
                        vps[jr][:],
                        wvt[:, kd, :],
                        xf[:, kd, jr * QCH : (jr + 1) * QCH],
                        start=(kd == 0),
                        stop=(kd == NKD - 1),
                    )
            nc.scalar.copy(out=kT[:, 0 : 2 * QCH], in_=kA[:])
            nc.scalar.copy(out=kT[:, 2 * QCH : S], in_=kB[:])
            rope(kT[:])
            for jr in range(NQC):
                nc.scalar.copy(
                    out=vT[:, jr * QCH : (jr + 1) * QCH], in_=vps[jr][:]
                )
            nc.sync.dma_start_transpose(vKf[:], vT[:])
            nc.scalar.copy(out=vK8[:], in_=vKf[:])

            # Q heads: even heads use banks A,B; odd heads use C..F so the
            # next head's matmuls never WAR-stall on the previous head's
            # PSUM->SBUF copies. The first jq0 attention group is emitted
            # between q1 and q2 so its exps overlap the q2/q3 projections.
            def proj_q(hh):
                if hh % 2 == 0:
                    qab = [psA("A"), psA("B")]
                    qdst = [
                        qab[jr // 2][:, (jr % 2) * QCH : (jr % 2 + 1) * QCH]
                        for jr in range(NQC)
                    ]
                else:
                    qcf = [psB(t) for t in "CDEF"]
                    qdst = [qcf[jr][:] for jr in range(NQC)]
                for kd in range(NKD):
                    for jr in range(NQC):
                        nc.tensor.matmul(
                            qdst[jr],
                            wqt[:, kd, hh * P : (hh + 1) * P],
                            xf[:, kd, jr * QCH : (jr + 1) * QCH],
                            start=(kd == 0),
                            stop=(kd == NKD - 1),
                        )
                if hh % 2 == 0:
                    nc.scalar.copy(out=qT[:, hh, 0 : 2 * QCH], in_=qab[0][:])
                    nc.scalar.copy(out=qT[:, hh, 2 * QCH : S], in_=qab[1][:])
                else:
                    for jr in range(NQC):
                        nc.scalar.copy(
                            out=qT[:, hh, jr * QCH : (jr + 1) * QCH],
                            in_=qcf[jr][:],
                        )
                rope(qT[:, hh, :])

            # ======== Fused attention + output projection ================
            # Two heads co-scheduled per group; sps double-buffers on the
            # global pair index so QK(next) overlaps exp(cur). The second
            # head walks its key pairs in reverse so the two heads' diagonal
            # (masked) pairs never land in the same slot (spreads DVE work).
            pair_it = [0]
            pending_units = []

            def next_ab():
                sps = psA("A" if pair_it[0] % 2 == 0 else "B")
                pair_it[0] += 1
                return sps

            def attn_group(ha, hb, jq):
                fp8 = jq >= 1
                nkc = 4 * (jq + 1)
                npair = nkc // 2
                ops = {ha: psB("C"), hb: psB("D")}
                dps = {ha: psB("E"), hb: psB("F")}
                qs = {
                    h: qT[:, h, jq * QCH : (jq + 1) * QCH] for h in (ha, hb)
                }
                seq = []
                for i in range(npair):
                    seq.append((ha, i))
                    seq.append((hb, npair - 1 - i))
                first = {ha: 0, hb: 1}
                last = {ha: len(seq) - 2, hb: len(seq) - 1}

                def emit_qk(h, ip):
                    sps = next_ab()
                    kc0 = 2 * ip
                    for k2 in range(2):
                        jd = kc0 + k2 - 4 * jq
                        diag = jd >= 0
                        half = sps[:, k2 * QCH : (k2 + 1) * QCH]
                        nc.tensor.matmul(
                            half,
                            kT[:, (kc0 + k2) * P : (kc0 + k2 + 1) * P],
                            qs[h],
                            start=True,
                            stop=not diag,
                        )
                        if diag:
                            ncols = min(QCH, P * (jd + 1))
                            nc.tensor.matmul(
                                half[:, :ncols],
                                tri_t[:],
                                mbias_t[:, jd, :ncols],
                                start=False,
                                stop=True,
                            )
                    return sps

                sps_cur = emit_qk(*seq[0])
                for i, (h, ip) in enumerate(seq):
                    kc0 = 2 * ip
                    if fp8:
                        pT = ppt.tile([P, 2, QCH], F8, tag="pT8")
                    else:
                        pT = ppt.tile([P, 2, QCH], F16, tag="pT16")
                    pflat = pT[:].rearrange("p a b -> p (a b)")
                    nc.scalar.activation(
                        pflat, sps_cur[:], AF.Exp, scale=SCALE, bias=eb[:]
                    )
                    if i + 1 < len(seq):
                        sps_cur = emit_qk(*seq[i + 1])
                    if fp8:
                        nc.tensor.matmul(
                            ops[h][:],
                            vK8[:, kc0 : kc0 + 2, :],
                            pT[:],
                            start=(i == first[h]),
                            stop=(i == last[h]),
                            perf_mode=DR,
                        )
                        nc.tensor.matmul(
                            dps[h][:],
                            ones8_t[:],
                            pT[:],
                            start=(i == first[h]),
                            stop=(i == last[h]),
                            perf_mode=DR,
                        )
                    else:
                        for k2 in range(2):
                            nc.tensor.matmul(
                                ops[h][:],
                                vKf[:, kc0 + k2, :],
                                pT[:, k2, :],
                                start=(i == first[h] and k2 == 0),
                                stop=(i == last[h] and k2 == 1),
                            )
                            nc.tensor.matmul(
                                dps[h][:],
                                ones16_t[:],
                                pT[:, k2, :],
                                start=(i == first[h] and k2 == 0),
                                stop=(i == last[h] and k2 == 1),
                            )
                    if i % 2 == 1 and pending_units:
                        pending_units.pop(0)()
                for h in (ha, hb):
                    dib = pdib.tile([P, QCH], F32, tag="dib")
                    nc.vector.reciprocal_approx_fast(dib[:], dps[h][:])
                    if fp8:
                        dst = attnT8[:, h, jq * QCH : (jq + 1) * QCH]
                    else:
                        dst = attnT16[:, h, :]
                    nc.vector.tensor_tensor(dst, ops[h][:], dib[:], OP.mult)

            def make_oproj_units(jq):
                units = []
                for op_i in range(D // P // 2):
                    def unit(op_i=op_i, jq=jq):
                        ps = next_ab()
                        for i2 in range(2):
                            oc = 2 * op_i + i2
                            half = ps[:, i2 * QCH : (i2 + 1) * QCH]
                            if jq == 0:
                                for a in range(NH):
                                    nc.tensor.matmul(
                                        half,
                                        wo16[:, a, oc * P : (oc + 1) * P],
                                        attnT16[:, a, :],
                                        start=(a == 0),
                                        stop=(a == NH - 1),
                                    )
                            else:
                                for a2 in range(0, NH, 2):
                                    nc.tensor.matmul(
                                        half,
                                        wo8[:, a2 : a2 + 2, oc * P : (oc + 1) * P],
                                        attnT8[
                                            :,
                                            a2 : a2 + 2,
                                            jq * QCH : (jq + 1) * QCH,
                                        ],
                                        start=(a2 == 0),
                                        stop=(a2 == NH - 2),
                                        perf_mode=DR,
                                    )
                        ot = p3.tile([P, 2 * QCH], F16, tag="ot")
                        tail = jq == NQC - 1 and op_i >= 6
                        if tail:
                            # last units: split halves across both engines so
                            # the copy+DMA tail drains in parallel
                            nc.scalar.copy(out=ot[:, :QCH], in_=ps[:, :QCH])
                            nc.vector.tensor_copy(
                                out=ot[:, QCH:], in_=ps[:, QCH:]
                            )
                        elif op_i % 2 == 0:
                            nc.scalar.copy(out=ot[:], in_=ps[:])
                        else:
                            nc.vector.tensor_copy(out=ot[:], in_=ps[:])
                        for i2 in range(2):
                            oc = 2 * op_i + i2
                            nc.sync.dma_start(
                                out_d[
                                    oc * P : (oc + 1) * P,
                                    jq * QCH : (jq + 1) * QCH,
                                ],
                                ot[:, i2 * QCH : (i2 + 1) * QCH],
                            )
                    units.append(unit)
                return units

            for hh in range(NH):
                proj_q(hh)
            for jq in range(NQC):
                attn_group(0, 1, jq)
                attn_group(2, 3, jq)
                pending_units.extend(make_oproj_units(jq))
            while pending_units:
                pending_units.pop(0)()

    nc.finalize()
    return nc


_NC = None


def _get_nc():
    global _NC
    if _NC is None:
        _NC = build_nc()
    return _NC


def make_in_maps(x, wq, wk, wv, wo):
    x = np.asarray(x, dtype=np.float32)
    f16 = np.float16
    in_maps = []
    for c in range(8):
        b, g = c // 4, c % 4
        in_maps.append(
            {
                "xT": np.ascontiguousarray(x[b].T).astype(f16),
                "wq": np.asarray(wq[:, QW * g : QW * (g + 1)], dtype=f16),
                "wk": np.asarray(wk[:, P * g : P * (g + 1)], dtype=f16),
                "wv": np.asarray(wv[:, P * g : P * (g + 1)], dtype=f16),
                "wo": np.asarray(wo[QW * g : QW * (g + 1), :], dtype=f16),
            }
        )
    return in_maps


def kernel(x, wq, wk, wv, wo):
    nc = _get_nc()
    in_maps = make_in_maps(x, wq, wk, wv, wo)
    res = run_bass_kernel_spmd(nc, in_maps, list(range(8)))
    parts = [res.results[c]["outT"].astype(np.float32) for c in range(8)]
    out = np.stack(
        [
            (parts[0] + parts[1] + parts[2] + parts[3]).T,
            (parts[4] + parts[5] + parts[6] + parts[7]).T,
        ]
    ).astype(np.float32)
    return out


# revision 32
# speedup vs baseline: 1.2027x; 1.0105x over previous
"""Causal GQA self-attention with RoPE for TRN2, 8 NeuronCores.

Problem: B=2, S=2048, D=2048, H=16 q-heads, KV=4 kv-heads, HD=128.

Sharding: core c = (batch b = c//4, kv-group g = c%4). Each core computes
q-heads 4g..4g+3 and kv-head g for batch b; host sums the 4 partial
output projections per batch and transposes back.

Perf structure (PE-bound kernel, ~47ns fixed cost + 0.42ns/col per matmul):
  - K/V projections stream kd-outer so PE starts as soon as the first
    x chunk lands (x DMA overlaps projection compute).
  - Attention and output projection are fused jq-major: O-proj for query
    chunk jq runs right after its 4 heads finish, spreading the output
    DMA across the whole attention phase.
  - fp8(e4m3) DoubleRow matmuls (2x PE throughput) for PV, the softmax
    denominator (ones-matmul), and O-proj on jq>=1 (rows with >=512 keys,
    where fp8 noise averages out); jq=0 rows (few keys; these dominate
    the output max) stay fp16 end to end.
  - exp computed as exp(s*scale - 1) so fp8 probabilities can't overflow;
    the e^-1 factor cancels in the normalize.
  - Output partials in fp16 (halves the output DMA; host sums in fp32).
"""
import sys

sys.path.insert(0, "/opt/trn_rl_repo")

import numpy as np
import ml_dtypes

import concourse.tile as tile
from concourse import bacc, mybir
from concourse.bass_utils import run_bass_kernel_spmd

F32 = mybir.dt.float32
F16 = mybir.dt.float16
F8 = mybir.dt.float8e4
DR = mybir.MatmulPerfMode.DoubleRow
AF = mybir.ActivationFunctionType
OP = mybir.AluOpType

P = 128          # partitions / head dim
S = 2048         # sequence length
D = 2048         # model dim
NH = 4           # q heads per core
QW = NH * P      # q projection width per core (512)
NKD = D // P     # contraction chunks (16)
QCH = 512        # query chunk (free dim of attention matmuls)
NQC = S // QCH   # 4
KCH = P          # key chunk (128, on partitions)
NKC = S // KCH   # 16
SCALE = float(P) ** -0.5


def _host_constants():
    inv = 1.0 / (10000.0 ** (np.arange(0, P, 2, dtype=np.float64) / P))  # [64]
    pos = np.arange(S, dtype=np.float64)
    freqs = pos[:, None] * inv[None, :]                  # [S, 64]
    emb = np.concatenate([freqs, freqs], axis=-1)        # [S, 128]
    cosT = np.cos(emb).T.astype(np.float16).copy()       # [128, S]
    sinT = np.sin(emb).T.astype(np.float16)
    sinT[: P // 2] *= np.float16(-1.0)                   # fold rotate_half sign
    sinT = sinT.copy()
    # Causal masking via PE: tri[c, kp] = [c <= kp] (stationary) and
    # mbias[c, jd, q] = -30000*([q == c + 128*jd - 1] + [c == 0][q < 128*jd - 1])
    # (moving) so that (tri.T @ mbias[:, jd]) adds -30000 exactly on the
    # masked positions {q < kp + 128*jd} of a diagonal key block.
    c = np.arange(P)
    kp = np.arange(P)
    tri = (c[:, None] <= kp[None, :]).astype(np.float16)          # [c, kp]
    q = np.arange(QCH)
    mbias = np.zeros((P, 4, QCH), dtype=np.float16)
    for jd in range(4):
        hit = (q[None, :] == c[:, None] + 128 * jd - 1)
        full = (c[:, None] == 0) & (q[None, :] < 128 * jd - 1)
        mbias[:, jd, :] = np.float16(-30000.0) * (hit | full)
    ones16 = np.ones((P, P), dtype=np.float16)
    ones8 = np.ones((P, 2 * P), dtype=ml_dtypes.float8_e4m3fn)
    return cosT, sinT, tri, mbias, ones16, ones8


def build_nc():
    cosT_np, sinT_np, tri_np, mbias_np, ones16_np, ones8_np = _host_constants()

    nc = bacc.Bacc(None)
    xT_d = nc.dram_tensor("xT", [D, S], F16, kind="ExternalInput")
    wq_d = nc.dram_tensor("wq", [D, QW], F16, kind="ExternalInput")
    wk_d = nc.dram_tensor("wk", [D, P], F16, kind="ExternalInput")
    wv_d = nc.dram_tensor("wv", [D, P], F16, kind="ExternalInput")
    wo_d = nc.dram_tensor("wo", [QW, D], F16, kind="ExternalInput")
    out_d = nc.dram_tensor("outT", [D, S], F16, kind="ExternalOutput")

    cos_d = nc.inline_tensor(cosT_np, name="cosT")
    sin_d = nc.inline_tensor(sinT_np, name="sinT")
    tri_d = nc.inline_tensor(tri_np, name="tri")
    mbias_d = nc.inline_tensor(mbias_np, name="mbias")
    ones16_d = nc.inline_tensor(ones16_np, name="ones16")
    ones8_d = nc.inline_tensor(ones8_np, name="ones8")

    # DRAM views with the contraction dim split for SBUF partitions.
    xT_v = xT_d[:].rearrange("(kd p) s -> p kd s", p=P)
    wq_v = wq_d[:].rearrange("(kd p) c -> p kd c", p=P)
    wk_v = wk_d[:].rearrange("(kd p) c -> p kd c", p=P)
    wv_v = wv_d[:].rearrange("(kd p) c -> p kd c", p=P)
    wo_v = wo_d[:].rearrange("(a p) o -> p a o", p=P)

    with tile.TileContext(nc) as tc:
        with tc.tile_pool(name="persist", bufs=1) as pp, \
             tc.tile_pool(name="pswap", bufs=2) as pswap, \
             tc.tile_pool(name="ppt", bufs=5) as ppt, \
             tc.tile_pool(name="pdib", bufs=3) as pdib, \
             tc.tile_pool(name="p3", bufs=4) as p3, \
             tc.tile_pool(name="ps", bufs=1, space="PSUM") as psp:
            qT = pp.tile([P, NH, S], F16)
            vT = pp.tile([P, S], F16)
            attnT16 = pp.tile([P, NH, QCH], F16)
            kT = pp.tile([P, S], F16)
            vKf = pp.tile([P, NKC, P], F16)      # V as (kpos, kchunk, hd)
            vK8 = pp.tile([P, NKC, P], F8)
            attnT8 = pp.tile([P, NH, S], F8)      # jq>=1 attention out
            wo16 = pp.tile([P, NH, D], F16)
            wo8 = pp.tile([P, NH, D], F8)
            tri_t = pp.tile([P, P], F16)
            mbias_t = pp.tile([P, 4, QCH], F16)
            ones16_t = pp.tile([P, P], F16)
            ones8_t = pp.tile([P, 2, P], F8)
            cos_t = pp.tile([P, S], F16)
            sin_t = pp.tile([P, S], F16)
            eb = pp.tile([P, 1], F32)
            nc.gpsimd.memset(eb[:], -1.0)

            wkt = pp.tile([P, NKD, P], F16)
            wvt = pp.tile([P, NKD, P], F16)
            wqt = pp.tile([P, NKD, QW], F16)
            xf = pp.tile([P, NKD, S], F16)

            # DMA priority order: first-needed first.
            nc.sync.dma_start(tri_t[:], tri_d[:])
            nc.sync.dma_start(wkt[:], wk_v)
            for jr in range(NQC):
                nc.sync.dma_start(
                    xf[:, 0, jr * QCH : (jr + 1) * QCH],
                    xT_v[:, 0, jr * QCH : (jr + 1) * QCH],
                )
            nc.sync.dma_start(wvt[:], wv_v)
            for kd in range(1, NKD):
                nc.sync.dma_start(xf[:, kd, :], xT_v[:, kd, :])
            nc.sync.dma_start(cos_t[:], cos_d[:])
            nc.sync.dma_start(sin_t[:], sin_d[:])
            nc.sync.dma_start(mbias_t[:], mbias_d[:])
            nc.sync.dma_start(ones16_t[:], ones16_d[:])
            nc.sync.dma_start(
                ones8_t[:], ones8_d[:].rearrange("p (a b) -> p a b", a=2)
            )
            nc.sync.dma_start(wqt[:], wq_v)
            nc.sync.dma_start(wo16[:], wo_v)
            nc.scalar.copy(out=wo8[:], in_=wo16[:])

            # PSUM tags: A,B = [128,1024] (2 banks each); C..F = [128,512].
            def psA(name):
                return psp.tile([P, 2 * QCH], F32, tag=name, name=name)

            def psB(name):
                return psp.tile([P, QCH], F32, tag=name, name=name)

            def rope(dst_ap):
                sw = pswap.tile([P, S], F16, tag="swap")
                half = P // 2
                nc.sync.dma_start(sw[:half, :], dst_ap[half:, :])
                nc.sync.dma_start(sw[half:, :], dst_ap[:half, :])
                nc.vector.tensor_tensor(sw[:], sw[:], sin_t[:], OP.mult)
                nc.vector.tensor_tensor(dst_ap, dst_ap, cos_t[:], OP.mult)
                nc.vector.tensor_tensor(dst_ap, dst_ap, sw[:], OP.add)

            # PE warmup while the first DMAs stream in: eb is memset on
            # device (no DMA dependency), so the PE can start ramping within
            # ~2us of kernel start; a few tri_t matmuls then keep it busy
            # until the first x chunk lands.
            wps = psB("C")
            for i in range(110):
                nc.tensor.matmul(
                    wps[0:1, 0:1], eb[:], eb[:], start=True, stop=True
                )
            for i in range(16):
                nc.tensor.matmul(
                    wps[:, 0:P], tri_t[:], tri_t[:], start=True, stop=True
                )

            # ======== Phase 1: QKV projections ==========================
            # K/V stream kd-outer (start on the first x chunk); K uses
            # banks A,B (1024-wide halves), V uses C..F.
            kA, kB = psA("A"), psA("B")
            vps = [psB(t) for t in "CDEF"]
            for kd in range(NKD):
                for jr in range(NQC):
                    dst = (kA, kB)[jr // 2][:, (jr % 2) * QCH : (jr % 2 + 1) * QCH]
                    nc.tensor.matmul(
                        dst,
                        wkt[:, kd, :],
                        xf[:, kd, jr * QCH : (jr + 1) * QCH],
                        start=(kd == 0),
                        stop=(kd == NKD - 1),
                    )
                for jr in range(NQC):
                    nc.tensor.matmul(
                        vps[jr][:],
                        wvt[:, kd, :],
                        xf[:, kd, jr * QCH : (jr + 1) * QCH],
                        start=(kd == 0),
                        stop=(kd == NKD - 1),
                    )
            nc.scalar.copy(out=kT[:, 0 : 2 * QCH], in_=kA[:])
            nc.scalar.copy(out=kT[:, 2 * QCH : S], in_=kB[:])
            rope(kT[:])
            for jr in range(NQC):
                nc.scalar.copy(
                    out=vT[:, jr * QCH : (jr + 1) * QCH], in_=vps[jr][:]
                )
            nc.sync.dma_start_transpose(vKf[:], vT[:])
            nc.scalar.copy(out=vK8[:], in_=vKf[:])

            # Q heads: even heads use banks A,B; odd heads use C..F so the
            # next head's matmuls never WAR-stall on the previous head's
            # PSUM->SBUF copies. The first jq0 attention group is emitted
            # between q1 and q2 so its exps overlap the q2/q3 projections.
            def proj_q(hh):
                if hh % 2 == 0:
                    qab = [psA("A"), psA("B")]
                    qdst = [
                        qab[jr // 2][:, (jr % 2) * QCH : (jr % 2 + 1) * QCH]
                        for jr in range(NQC)
                    ]
                else:
                    qcf = [psB(t) for t in "CDEF"]
                    qdst = [qcf[jr][:] for jr in range(NQC)]
                for kd in range(NKD):
                    for jr in range(NQC):
                        nc.tensor.matmul(
                            qdst[jr],
                            wqt[:, kd, hh * P : (hh + 1) * P],
                            xf[:, kd, jr * QCH : (jr + 1) * QCH],
                            start=(kd == 0),
                            stop=(kd == NKD - 1),
                        )
                if hh % 2 == 0:
                    nc.scalar.copy(out=qT[:, hh, 0 : 2 * QCH], in_=qab[0][:])
                    nc.scalar.copy(out=qT[:, hh, 2 * QCH : S], in_=qab[1][:])
                else:
                    for jr in range(NQC):
                        nc.scalar.copy(
                            out=qT[:, hh, jr * QCH : (jr + 1) * QCH],
                            in_=qcf[jr][:],
                        )
                rope(qT[:, hh, :])

            # ======== Fused attention + output projection ================
            # Two heads co-scheduled per group; sps double-buffers on the
            # global pair index so QK(next) overlaps exp(cur). The second
            # head walks its key pairs in reverse so the two heads' diagonal
            # (masked) pairs never land in the same slot (spreads DVE work).
            pair_it = [0]
            pending_units = []

            def next_ab():
                sps = psA("A" if pair_it[0] % 2 == 0 else "B")
                pair_it[0] += 1
                return sps

            def attn_group(ha, hb, jq):
                fp8 = jq >= 1
                nkc = 4 * (jq + 1)
                npair = nkc // 2
                ops = {ha: psB("C"), hb: psB("D")}
                dps = {ha: psB("E"), hb: psB("F")}
                qs = {
                    h: qT[:, h, jq * QCH : (jq + 1) * QCH] for h in (ha, hb)
                }
                seq = []
                for i in range(npair):
                    seq.append((ha, i))
                    seq.append((hb, npair - 1 - i))
                first = {ha: 0, hb: 1}
                last = {ha: len(seq) - 2, hb: len(seq) - 1}

                def emit_qk(h, ip):
                    sps = next_ab()
                    kc0 = 2 * ip
                    for k2 in range(2):
                        jd = kc0 + k2 - 4 * jq
                        diag = jd >= 0
                        half = sps[:, k2 * QCH : (k2 + 1) * QCH]
                        nc.tensor.matmul(
                            half,
                            kT[:, (kc0 + k2) * P : (kc0 + k2 + 1) * P],
                            qs[h],
                            start=True,
                            stop=not diag,
                        )
                        if diag:
                            ncols = min(QCH, P * (jd + 1))
                            nc.tensor.matmul(
                                half[:, :ncols],
                                tri_t[:],
                                mbias_t[:, jd, :ncols],
                                start=False,
                                stop=True,
                            )
                    return sps

                sps_cur = emit_qk(*seq[0])
                for i, (h, ip) in enumerate(seq):
                    kc0 = 2 * ip
                    if fp8:
                        pT = ppt.tile([P, 2, QCH], F8, tag="pT8")
                    else:
                        pT = ppt.tile([P, 2, QCH], F16, tag="pT16")
                    pflat = pT[:].rearrange("p a b -> p (a b)")
                    nc.scalar.activation(
                        pflat, sps_cur[:], AF.Exp, scale=SCALE, bias=eb[:]
                    )
                    if i + 1 < len(seq):
                        sps_cur = emit_qk(*seq[i + 1])
                    if fp8:
                        nc.tensor.matmul(
                            ops[h][:],
                            vK8[:, kc0 : kc0 + 2, :],
                            pT[:],
                            start=(i == first[h]),
                            stop=(i == last[h]),
                            perf_mode=DR,
                        )
                        nc.tensor.matmul(
                            dps[h][:],
                            ones8_t[:],
                            pT[:],
                            start=(i == first[h]),
                            stop=(i == last[h]),
                            perf_mode=DR,
                        )
                    else:
                        for k2 in range(2):
                            nc.tensor.matmul(
                                ops[h][:],
                                vKf[:, kc0 + k2, :],
                                pT[:, k2, :],
                                start=(i == first[h] and k2 == 0),
                                stop=(i == last[h] and k2 == 1),
                            )
                            nc.tensor.matmul(
                                dps[h][:],
                                ones16_t[:],
                                pT[:, k2, :],
                                start=(i == first[h] and k2 == 0),
                                stop=(i == last[h] and k2 == 1),
                            )
                    if i % 2 == 1 and pending_units:
                        pending_units.pop(0)()
                for h in (ha, hb):
                    dib = pdib.tile([P, QCH], F32, tag="dib")
                    nc.vector.reciprocal_approx_fast(dib[:], dps[h][:])
                    if fp8:
                        dst = attnT8[:, h, jq * QCH : (jq + 1) * QCH]
                    else:
                        dst = attnT16[:, h, :]
                    nc.vector.tensor_tensor(dst, ops[h][:], dib[:], OP.mult)

            def make_oproj_units(jq):
                units = []
                for op_i in range(D // P // 2):
                    def unit(op_i=op_i, jq=jq):
                        ps = next_ab()
                        for i2 in range(2):
                            oc = 2 * op_i + i2
                            half = ps[:, i2 * QCH : (i2 + 1) * QCH]
                            if jq == 0:
                                for a in range(NH):
                                    nc.tensor.matmul(
                                        half,
                                        wo16[:, a, oc * P : (oc + 1) * P],
                                        attnT16[:, a, :],
                                        start=(a == 0),
                                        stop=(a == NH - 1),
                                    )
                            else:
                                for a2 in range(0, NH, 2):
                                    nc.tensor.matmul(
                                        half,
                                        wo8[:, a2 : a2 + 2, oc * P : (oc + 1) * P],
                                        attnT8[
                                            :,
                                            a2 : a2 + 2,
                                            jq * QCH : (jq + 1) * QCH,
                                        ],
                                        start=(a2 == 0),
                                        stop=(a2 == NH - 2),
                                        perf_mode=DR,
                                    )
                        ot = p3.tile([P, 2 * QCH], F16, tag="ot")
                        tail = jq == NQC - 1 and op_i >= 6
                        if tail:
                            # last units: split halves across both engines so
                            # the copy+DMA tail drains in parallel
                            nc.scalar.copy(out=ot[:, :QCH], in_=ps[:, :QCH])
                            nc.vector.tensor_copy(
                                out=ot[:, QCH:], in_=ps[:, QCH:]
                            )
                        elif op_i % 2 == 0:
                            nc.scalar.copy(out=ot[:], in_=ps[:])
                        else:
                            nc.vector.tensor_copy(out=ot[:], in_=ps[:])
                        for i2 in range(2):
                            oc = 2 * op_i + i2
                            nc.sync.dma_start(
                                out_d[
                                    oc * P : (oc + 1) * P,
                                    jq * QCH : (jq + 1) * QCH,
                                ],
                                ot[:, i2 * QCH : (i2 + 1) * QCH],
                            )
                    units.append(unit)
                return units

            for hh in range(NH):
                proj_q(hh)
            for jq in range(NQC):
                attn_group(0, 1, jq)
                attn_group(2, 3, jq)
                pending_units.extend(make_oproj_units(jq))
            while pending_units:
                pending_units.pop(0)()

    nc.finalize()
    return nc


_NC = None


def _get_nc():
    global _NC
    if _NC is None:
        _NC = build_nc()
    return _NC


def make_in_maps(x, wq, wk, wv, wo):
    x = np.asarray(x, dtype=np.float32)
    f16 = np.float16
    in_maps = []
    for c in range(8):
        b, g = c // 4, c % 4
        in_maps.append(
            {
                "xT": np.ascontiguousarray(x[b].T).astype(f16),
                "wq": np.asarray(wq[:, QW * g : QW * (g + 1)], dtype=f16),
                "wk": np.asarray(wk[:, P * g : P * (g + 1)], dtype=f16),
                "wv": np.asarray(wv[:, P * g : P * (g + 1)], dtype=f16),
                "wo": np.asarray(wo[QW * g : QW * (g + 1), :], dtype=f16),
            }
        )
    return in_maps


def kernel(x, wq, wk, wv, wo):
    nc = _get_nc()
    in_maps = make_in_maps(x, wq, wk, wv, wo)
    res = run_bass_kernel_spmd(nc, in_maps, list(range(8)))
    parts = [res.results[c]["outT"].astype(np.float32) for c in range(8)]
    out = np.stack(
        [
            (parts[0] + parts[1] + parts[2] + parts[3]).T,
            (parts[4] + parts[5] + parts[6] + parts[7]).T,
        ]
    ).astype(np.float32)
    return out


# revision 34
# speedup vs baseline: 1.2100x; 1.0061x over previous
"""Causal GQA self-attention with RoPE for TRN2, 8 NeuronCores.

Problem: B=2, S=2048, D=2048, H=16 q-heads, KV=4 kv-heads, HD=128.

Sharding: core c = (batch b = c//4, kv-group g = c%4). Each core computes
q-heads 4g..4g+3 and kv-head g for batch b; host sums the 4 partial
output projections per batch and transposes back.

Perf structure (PE-bound kernel, ~47ns fixed cost + 0.42ns/col per matmul):
  - K/V projections stream kd-outer so PE starts as soon as the first
    x chunk lands (x DMA overlaps projection compute).
  - Attention and output projection are fused jq-major: O-proj for query
    chunk jq runs right after its 4 heads finish, spreading the output
    DMA across the whole attention phase.
  - fp8(e4m3) DoubleRow matmuls (2x PE throughput) for PV, the softmax
    denominator (ones-matmul), and O-proj on jq>=1 (rows with >=512 keys,
    where fp8 noise averages out); jq=0 rows (few keys; these dominate
    the output max) stay fp16 end to end.
  - exp computed as exp(s*scale - 1) so fp8 probabilities can't overflow;
    the e^-1 factor cancels in the normalize.
  - Output partials in fp16 (halves the output DMA; host sums in fp32).
"""
import sys

sys.path.insert(0, "/opt/trn_rl_repo")

import numpy as np
import ml_dtypes

import concourse.tile as tile
from concourse import bacc, mybir
from concourse.bass_utils import run_bass_kernel_spmd

F32 = mybir.dt.float32
F16 = mybir.dt.float16
F8 = mybir.dt.float8e4
DR = mybir.MatmulPerfMode.DoubleRow
AF = mybir.ActivationFunctionType
OP = mybir.AluOpType

P = 128          # partitions / head dim
S = 2048         # sequence length
D = 2048         # model dim
NH = 4           # q heads per core
QW = NH * P      # q projection width per core (512)
NKD = D // P     # contraction chunks (16)
QCH = 512        # query chunk (free dim of attention matmuls)
NQC = S // QCH   # 4
KCH = P          # key chunk (128, on partitions)
NKC = S // KCH   # 16
SCALE = float(P) ** -0.5


def _host_constants():
    inv = 1.0 / (10000.0 ** (np.arange(0, P, 2, dtype=np.float64) / P))  # [64]
    pos = np.arange(S, dtype=np.float64)
    freqs = pos[:, None] * inv[None, :]                  # [S, 64]
    emb = np.concatenate([freqs, freqs], axis=-1)        # [S, 128]
    cosT = np.cos(emb).T.astype(np.float16).copy()       # [128, S]
    sinT = np.sin(emb).T.astype(np.float16)
    sinT[: P // 2] *= np.float16(-1.0)                   # fold rotate_half sign
    sinT = sinT.copy()
    # Causal masking via PE: tri[c, kp] = [c <= kp] (stationary) and
    # mbias[c, jd, q] = -30000*([q == c + 128*jd - 1] + [c == 0][q < 128*jd - 1])
    # (moving) so that (tri.T @ mbias[:, jd]) adds -30000 exactly on the
    # masked positions {q < kp + 128*jd} of a diagonal key block.
    c = np.arange(P)
    kp = np.arange(P)
    tri = (c[:, None] <= kp[None, :]).astype(np.float16)          # [c, kp]
    q = np.arange(QCH)
    mbias = np.zeros((P, 4, QCH), dtype=np.float16)
    for jd in range(4):
        hit = (q[None, :] == c[:, None] + 128 * jd - 1)
        full = (c[:, None] == 0) & (q[None, :] < 128 * jd - 1)
        mbias[:, jd, :] = np.float16(-30000.0) * (hit | full)
    ones16 = np.ones((P, P), dtype=np.float16)
    ones8 = np.ones((P, 2 * P), dtype=ml_dtypes.float8_e4m3fn)
    return cosT, sinT, tri, mbias, ones16, ones8


def build_nc():
    cosT_np, sinT_np, tri_np, mbias_np, ones16_np, ones8_np = _host_constants()

    nc = bacc.Bacc(None)
    xT_d = nc.dram_tensor("xT", [D, S], F16, kind="ExternalInput")
    wq_d = nc.dram_tensor("wq", [D, QW], F16, kind="ExternalInput")
    wk_d = nc.dram_tensor("wk", [D, P], F16, kind="ExternalInput")
    wv_d = nc.dram_tensor("wv", [D, P], F16, kind="ExternalInput")
    wo_d = nc.dram_tensor("wo", [QW, D], F16, kind="ExternalInput")
    out_d = nc.dram_tensor("outT", [D, S], F16, kind="ExternalOutput")

    cos_d = nc.inline_tensor(cosT_np, name="cosT")
    sin_d = nc.inline_tensor(sinT_np, name="sinT")
    tri_d = nc.inline_tensor(tri_np, name="tri")
    mbias_d = nc.inline_tensor(mbias_np, name="mbias")
    ones16_d = nc.inline_tensor(ones16_np, name="ones16")
    ones8_d = nc.inline_tensor(ones8_np, name="ones8")

    # DRAM views with the contraction dim split for SBUF partitions.
    xT_v = xT_d[:].rearrange("(kd p) s -> p kd s", p=P)
    wq_v = wq_d[:].rearrange("(kd p) c -> p kd c", p=P)
    wk_v = wk_d[:].rearrange("(kd p) c -> p kd c", p=P)
    wv_v = wv_d[:].rearrange("(kd p) c -> p kd c", p=P)
    wo_v = wo_d[:].rearrange("(a p) o -> p a o", p=P)

    with tile.TileContext(nc) as tc:
        with tc.tile_pool(name="persist", bufs=1) as pp, \
             tc.tile_pool(name="pswap", bufs=2) as pswap, \
             tc.tile_pool(name="ppt", bufs=4) as ppt, \
             tc.tile_pool(name="pdib", bufs=2) as pdib, \
             tc.tile_pool(name="p3", bufs=4) as p3, \
             tc.tile_pool(name="ps", bufs=1, space="PSUM") as psp:
            qT = pp.tile([P, NH, S], F16)
            vT = pp.tile([P, S], F16)
            attnT16 = pp.tile([P, NH, QCH], F16)
            kT = pp.tile([P, S], F16)
            vKf = pp.tile([P, NKC, P], F16)      # V as (kpos, kchunk, hd)
            vK8 = pp.tile([P, NKC, P], F8)
            attnT8 = pp.tile([P, NH, S], F8)      # jq>=1 attention out
            wo16 = pp.tile([P, NH, D], F16)
            wo8 = pp.tile([P, NH, D], F8)
            tri_t = pp.tile([P, P], F16)
            mbias_t = pp.tile([P, 4, QCH], F16)
            ones16_t = pp.tile([P, P], F16)
            ones8_t = pp.tile([P, 2, P], F8)
            cos_t = pp.tile([P, S], F16)
            sin_t = pp.tile([P, S], F16)
            eb = pp.tile([P, 1], F32)
            nc.gpsimd.memset(eb[:], -1.0)

            wkt = pp.tile([P, NKD, P], F16)
            wvt = pp.tile([P, NKD, P], F16)
            wqt = pp.tile([P, NKD, QW], F16)
            xf = pp.tile([P, NKD, S], F16)

            # DMA priority order: first-needed first.
            nc.sync.dma_start(tri_t[:], tri_d[:])
            nc.sync.dma_start(wkt[:], wk_v)
            for jr in range(NQC):
                nc.sync.dma_start(
                    xf[:, 0, jr * QCH : (jr + 1) * QCH],
                    xT_v[:, 0, jr * QCH : (jr + 1) * QCH],
                )
            nc.sync.dma_start(wvt[:], wv_v)
            for kd in range(1, NKD):
                nc.sync.dma_start(xf[:, kd, :], xT_v[:, kd, :])
            nc.sync.dma_start(cos_t[:], cos_d[:])
            nc.sync.dma_start(sin_t[:], sin_d[:])
            nc.sync.dma_start(mbias_t[:], mbias_d[:])
            nc.sync.dma_start(ones16_t[:], ones16_d[:])
            nc.sync.dma_start(
                ones8_t[:], ones8_d[:].rearrange("p (a b) -> p a b", a=2)
            )
            nc.sync.dma_start(wqt[:], wq_v)
            nc.sync.dma_start(wo16[:], wo_v)
            nc.scalar.copy(out=wo8[:], in_=wo16[:])

            # PSUM tags: A,B = [128,1024] (2 banks each); C..F = [128,512].
            def psA(name):
                return psp.tile([P, 2 * QCH], F32, tag=name, name=name)

            def psB(name):
                return psp.tile([P, QCH], F32, tag=name, name=name)

            def rope(dst_ap):
                sw = pswap.tile([P, S], F16, tag="swap")
                half = P // 2
                nc.sync.dma_start(sw[:half, :], dst_ap[half:, :])
                nc.sync.dma_start(sw[half:, :], dst_ap[:half, :])
                nc.vector.tensor_tensor(sw[:], sw[:], sin_t[:], OP.mult)
                nc.vector.tensor_tensor(dst_ap, dst_ap, cos_t[:], OP.mult)
                nc.vector.tensor_tensor(dst_ap, dst_ap, sw[:], OP.add)

            # PE warmup while the first DMAs stream in: eb is memset on
            # device (no DMA dependency), so the PE can start ramping within
            # ~2us of kernel start; a few tri_t matmuls then keep it busy
            # until the first x chunk lands.
            wps = psB("C")
            for i in range(110):
                nc.tensor.matmul(
                    wps[0:1, 0:1], eb[:], eb[:], start=True, stop=True
                )
            for i in range(24):
                nc.tensor.matmul(
                    wps[:, 0:P], tri_t[:], tri_t[:], start=True, stop=True
                )

            # ======== Phase 1: QKV projections ==========================
            # K/V stream kd-outer (start on the first x chunk); K uses
            # banks A,B (1024-wide halves), V uses C..F.
            kA, kB = psA("A"), psA("B")
            vps = [psB(t) for t in "CDEF"]
            for kd in range(NKD):
                for jr in range(NQC):
                    dst = (kA, kB)[jr // 2][:, (jr % 2) * QCH : (jr % 2 + 1) * QCH]
                    nc.tensor.matmul(
                        dst,
                        wkt[:, kd, :],
                        xf[:, kd, jr * QCH : (jr + 1) * QCH],
                        start=(kd == 0),
                        stop=(kd == NKD - 1),
                    )
                for jr in range(NQC):
                    nc.tensor.matmul(
                        vps[jr][:],
                        wvt[:, kd, :],
                        xf[:, kd, jr * QCH : (jr + 1) * QCH],
                        start=(kd == 0),
                        stop=(kd == NKD - 1),
                    )
            nc.scalar.copy(out=kT[:, 0 : 2 * QCH], in_=kA[:])
            nc.scalar.copy(out=kT[:, 2 * QCH : S], in_=kB[:])
            rope(kT[:])
            for jr in range(NQC):
                nc.scalar.copy(
                    out=vT[:, jr * QCH : (jr + 1) * QCH], in_=vps[jr][:]
                )
            nc.sync.dma_start_transpose(vKf[:], vT[:])
            nc.scalar.copy(out=vK8[:], in_=vKf[:])

            # Q heads: even heads use banks A,B; odd heads use C..F so the
            # next head's matmuls never WAR-stall on the previous head's
            # PSUM->SBUF copies. The first jq0 attention group is emitted
            # between q1 and q2 so its exps overlap the q2/q3 projections.
            def proj_q(hh):
                if hh % 2 == 0:
                    qab = [psA("A"), psA("B")]
                    qdst = [
                        qab[jr // 2][:, (jr % 2) * QCH : (jr % 2 + 1) * QCH]
                        for jr in range(NQC)
                    ]
                else:
                    qcf = [psB(t) for t in "CDEF"]
                    qdst = [qcf[jr][:] for jr in range(NQC)]
                for kd in range(NKD):
                    for jr in range(NQC):
                        nc.tensor.matmul(
                            qdst[jr],
                            wqt[:, kd, hh * P : (hh + 1) * P],
                            xf[:, kd, jr * QCH : (jr + 1) * QCH],
                            start=(kd == 0),
                            stop=(kd == NKD - 1),
                        )
                if hh % 2 == 0:
                    nc.scalar.copy(out=qT[:, hh, 0 : 2 * QCH], in_=qab[0][:])
                    nc.scalar.copy(out=qT[:, hh, 2 * QCH : S], in_=qab[1][:])
                else:
                    for jr in range(NQC):
                        nc.scalar.copy(
                            out=qT[:, hh, jr * QCH : (jr + 1) * QCH],
                            in_=qcf[jr][:],
                        )
                rope(qT[:, hh, :])

            # ======== Fused attention + output projection ================
            # Two heads co-scheduled per group; sps double-buffers on the
            # global pair index so QK(next) overlaps exp(cur). The second
            # head walks its key pairs in reverse so the two heads' diagonal
            # (masked) pairs never land in the same slot (spreads DVE work).
            pair_it = [0]
            pending_units = []

            def next_ab():
                sps = psA("A" if pair_it[0] % 2 == 0 else "B")
                pair_it[0] += 1
                return sps

            def attn_group(ha, hb, jq):
                fp8 = jq >= 1
                nkc = 4 * (jq + 1)
                npair = nkc // 2
                ops = {ha: psB("C"), hb: psB("D")}
                dps = {ha: psB("E"), hb: psB("F")}
                qs = {
                    h: qT[:, h, jq * QCH : (jq + 1) * QCH] for h in (ha, hb)
                }
                seq = []
                for i in range(npair):
                    seq.append((ha, i))
                    seq.append((hb, npair - 1 - i))
                first = {ha: 0, hb: 1}
                last = {ha: len(seq) - 2, hb: len(seq) - 1}

                def emit_qk(h, ip):
                    sps = next_ab()
                    kc0 = 2 * ip
                    for k2 in range(2):
                        jd = kc0 + k2 - 4 * jq
                        diag = jd >= 0
                        half = sps[:, k2 * QCH : (k2 + 1) * QCH]
                        nc.tensor.matmul(
                            half,
                            kT[:, (kc0 + k2) * P : (kc0 + k2 + 1) * P],
                            qs[h],
                            start=True,
                            stop=not diag,
                        )
                        if diag:
                            ncols = min(QCH, P * (jd + 1))
                            nc.tensor.matmul(
                                half[:, :ncols],
                                tri_t[:],
                                mbias_t[:, jd, :ncols],
                                start=False,
                                stop=True,
                            )
                    return sps

                sps_cur = emit_qk(*seq[0])
                for i, (h, ip) in enumerate(seq):
                    kc0 = 2 * ip
                    if fp8:
                        pT = ppt.tile([P, 2, QCH], F8, tag="pT8")
                    else:
                        pT = ppt.tile([P, 2, QCH], F16, tag="pT16")
                    pflat = pT[:].rearrange("p a b -> p (a b)")
                    nc.scalar.activation(
                        pflat, sps_cur[:], AF.Exp, scale=SCALE, bias=eb[:]
                    )
                    if i + 1 < len(seq):
                        sps_cur = emit_qk(*seq[i + 1])
                    if fp8:
                        nc.tensor.matmul(
                            ops[h][:],
                            vK8[:, kc0 : kc0 + 2, :],
                            pT[:],
                            start=(i == first[h]),
                            stop=(i == last[h]),
                            perf_mode=DR,
                        )
                        nc.tensor.matmul(
                            dps[h][:],
                            ones8_t[:],
                            pT[:],
                            start=(i == first[h]),
                            stop=(i == last[h]),
                            perf_mode=DR,
                        )
                    else:
                        for k2 in range(2):
                            nc.tensor.matmul(
                                ops[h][:],
                                vKf[:, kc0 + k2, :],
                                pT[:, k2, :],
                                start=(i == first[h] and k2 == 0),
                                stop=(i == last[h] and k2 == 1),
                            )
                            nc.tensor.matmul(
                                dps[h][:],
                                ones16_t[:],
                                pT[:, k2, :],
                                start=(i == first[h] and k2 == 0),
                                stop=(i == last[h] and k2 == 1),
                            )
                    if i % 2 == 1 and pending_units:
                        pending_units.pop(0)()
                for h in (ha, hb):
                    dib = pdib.tile([P, QCH], F32, tag="dib")
                    nc.vector.reciprocal_approx_fast(dib[:], dps[h][:])
                    if fp8:
                        dst = attnT8[:, h, jq * QCH : (jq + 1) * QCH]
                    else:
                        dst = attnT16[:, h, :]
                    nc.vector.tensor_tensor(dst, ops[h][:], dib[:], OP.mult)

            def make_oproj_units(jq):
                units = []
                for op_i in range(D // P // 2):
                    def unit(op_i=op_i, jq=jq):
                        ps = next_ab()
                        for i2 in range(2):
                            oc = 2 * op_i + i2
                            half = ps[:, i2 * QCH : (i2 + 1) * QCH]
                            if jq == 0:
                                for a in range(NH):
                                    nc.tensor.matmul(
                                        half,
                                        wo16[:, a, oc * P : (oc + 1) * P],
                                        attnT16[:, a, :],
                                        start=(a == 0),
                                        stop=(a == NH - 1),
                                    )
                            else:
                                for a2 in range(0, NH, 2):
                                    nc.tensor.matmul(
                                        half,
                                        wo8[:, a2 : a2 + 2, oc * P : (oc + 1) * P],
                                        attnT8[
                                            :,
                                            a2 : a2 + 2,
                                            jq * QCH : (jq + 1) * QCH,
                                        ],
                                        start=(a2 == 0),
                                        stop=(a2 == NH - 2),
                                        perf_mode=DR,
                                    )
                        ot = p3.tile([P, 2 * QCH], F16, tag="ot")
                        tail = jq == NQC - 1 and op_i >= 4
                        if tail:
                            # last units: split halves across both engines so
                            # the copy+DMA tail drains in parallel
                            nc.scalar.copy(out=ot[:, :QCH], in_=ps[:, :QCH])
                            nc.vector.tensor_copy(
                                out=ot[:, QCH:], in_=ps[:, QCH:]
                            )
                        elif op_i % 2 == 0:
                            nc.scalar.copy(out=ot[:], in_=ps[:])
                        else:
                            nc.vector.tensor_copy(out=ot[:], in_=ps[:])
                        for i2 in range(2):
                            oc = 2 * op_i + i2
                            nc.sync.dma_start(
                                out_d[
                                    oc * P : (oc + 1) * P,
                                    jq * QCH : (jq + 1) * QCH,
                                ],
                                ot[:, i2 * QCH : (i2 + 1) * QCH],
                            )
                    units.append(unit)
                return units

            for hh in range(NH):
                proj_q(hh)
            for jq in range(NQC):
                attn_group(0, 1, jq)
                attn_group(2, 3, jq)
                pending_units.extend(make_oproj_units(jq))
            while pending_units:
                pending_units.pop(0)()

    nc.finalize()
    return nc


_NC = None


def _get_nc():
    global _NC
    if _NC is None:
        _NC = build_nc()
    return _NC


def make_in_maps(x, wq, wk, wv, wo):
    x = np.asarray(x, dtype=np.float32)
    f16 = np.float16
    in_maps = []
    for c in range(8):
        b, g = c // 4, c % 4
        in_maps.append(
            {
                "xT": np.ascontiguousarray(x[b].T).astype(f16),
                "wq": np.asarray(wq[:, QW * g : QW * (g + 1)], dtype=f16),
                "wk": np.asarray(wk[:, P * g : P * (g + 1)], dtype=f16),
                "wv": np.asarray(wv[:, P * g : P * (g + 1)], dtype=f16),
                "wo": np.asarray(wo[QW * g : QW * (g + 1), :], dtype=f16),
            }
        )
    return in_maps


def kernel(x, wq, wk, wv, wo):
    nc = _get_nc()
    in_maps = make_in_maps(x, wq, wk, wv, wo)
    res = run_bass_kernel_spmd(nc, in_maps, list(range(8)))
    parts = [res.results[c]["outT"].astype(np.float32) for c in range(8)]
    out = np.stack(
        [
            (parts[0] + parts[1] + parts[2] + parts[3]).T,
            (parts[4] + parts[5] + parts[6] + parts[7]).T,
        ]
    ).astype(np.float32)
    return out


# revision 36
# speedup vs baseline: 1.2368x; 1.0221x over previous
"""Causal GQA self-attention with RoPE for TRN2, 8 NeuronCores.

Problem: B=2, S=2048, D=2048, H=16 q-heads, KV=4 kv-heads, HD=128.

Sharding: core c = (batch b = c//4, kv-group g = c%4). Each core computes
q-heads 4g..4g+3 and kv-head g for batch b; host sums the 4 partial
output projections per batch and transposes back.

Perf structure (PE-bound kernel, ~47ns fixed cost + 0.42ns/col per matmul):
  - K/V projections stream kd-outer so PE starts as soon as the first
    x chunk lands (x DMA overlaps projection compute).
  - Attention and output projection are fused jq-major: O-proj for query
    chunk jq runs right after its 4 heads finish, spreading the output
    DMA across the whole attention phase.
  - fp8(e4m3) DoubleRow matmuls (2x PE throughput) for PV, the softmax
    denominator (ones-matmul), and O-proj on jq>=1 (rows with >=512 keys,
    where fp8 noise averages out); jq=0 rows (few keys; these dominate
    the output max) stay fp16 end to end.
  - exp computed as exp(s*scale - 1) so fp8 probabilities can't overflow;
    the e^-1 factor cancels in the normalize.
  - Output partials in fp16 (halves the output DMA; host sums in fp32).
"""
import sys

sys.path.insert(0, "/opt/trn_rl_repo")

import numpy as np
import ml_dtypes

import concourse.tile as tile
from concourse import bacc, mybir
from concourse.bass_utils import run_bass_kernel_spmd

F32 = mybir.dt.float32
F16 = mybir.dt.float16
F8 = mybir.dt.float8e4
DR = mybir.MatmulPerfMode.DoubleRow
AF = mybir.ActivationFunctionType
OP = mybir.AluOpType

P = 128          # partitions / head dim
S = 2048         # sequence length
D = 2048         # model dim
NH = 4           # q heads per core
QW = NH * P      # q projection width per core (512)
NKD = D // P     # contraction chunks (16)
QCH = 512        # query chunk (free dim of attention matmuls)
NQC = S // QCH   # 4
KCH = P          # key chunk (128, on partitions)
NKC = S // KCH   # 16
SCALE = float(P) ** -0.5


def _host_constants():
    inv = 1.0 / (10000.0 ** (np.arange(0, P, 2, dtype=np.float64) / P))  # [64]
    pos = np.arange(S, dtype=np.float64)
    freqs = pos[:, None] * inv[None, :]                  # [S, 64]
    emb = np.concatenate([freqs, freqs], axis=-1)        # [S, 128]
    cosT = np.cos(emb).T.astype(np.float16).copy()       # [128, S]
    sinT = np.sin(emb).T.astype(np.float16)
    sinT[: P // 2] *= np.float16(-1.0)                   # fold rotate_half sign
    sinT = sinT.copy()
    # Causal masking via PE: tri[c, kp] = [c <= kp] (stationary) and
    # mbias[c, jd, q] = -30000*([q == c + 128*jd - 1] + [c == 0][q < 128*jd - 1])
    # (moving) so that (tri.T @ mbias[:, jd]) adds -30000 exactly on the
    # masked positions {q < kp + 128*jd} of a diagonal key block.
    c = np.arange(P)
    kp = np.arange(P)
    tri = (c[:, None] <= kp[None, :]).astype(np.float16)          # [c, kp]
    q = np.arange(QCH)
    mbias = np.zeros((P, 4, QCH), dtype=np.float16)
    for jd in range(4):
        hit = (q[None, :] == c[:, None] + 128 * jd - 1)
        full = (c[:, None] == 0) & (q[None, :] < 128 * jd - 1)
        mbias[:, jd, :] = np.float16(-30000.0) * (hit | full)
    ones16 = np.ones((P, P), dtype=np.float16)
    ones8 = np.ones((P, 2 * P), dtype=ml_dtypes.float8_e4m3fn)
    return cosT, sinT, tri, mbias, ones16, ones8


def build_nc():
    cosT_np, sinT_np, tri_np, mbias_np, ones16_np, ones8_np = _host_constants()

    nc = bacc.Bacc(None)
    xT_d = nc.dram_tensor("xT", [D, S], F16, kind="ExternalInput")
    wq_d = nc.dram_tensor("wq", [D, QW], F16, kind="ExternalInput")
    wk_d = nc.dram_tensor("wk", [D, P], F16, kind="ExternalInput")
    wv_d = nc.dram_tensor("wv", [D, P], F16, kind="ExternalInput")
    wo_d = nc.dram_tensor("wo", [QW, D], F16, kind="ExternalInput")
    out_d = nc.dram_tensor("outT", [D, S], F16, kind="ExternalOutput")

    cos_d = nc.inline_tensor(cosT_np, name="cosT")
    sin_d = nc.inline_tensor(sinT_np, name="sinT")
    tri_d = nc.inline_tensor(tri_np, name="tri")
    mbias_d = nc.inline_tensor(mbias_np, name="mbias")
    ones16_d = nc.inline_tensor(ones16_np, name="ones16")
    ones8_d = nc.inline_tensor(ones8_np, name="ones8")

    # DRAM views with the contraction dim split for SBUF partitions.
    xT_v = xT_d[:].rearrange("(kd p) s -> p kd s", p=P)
    wq_v = wq_d[:].rearrange("(kd p) c -> p kd c", p=P)
    wk_v = wk_d[:].rearrange("(kd p) c -> p kd c", p=P)
    wv_v = wv_d[:].rearrange("(kd p) c -> p kd c", p=P)
    wo_v = wo_d[:].rearrange("(a p) o -> p a o", p=P)

    with tile.TileContext(nc) as tc:
        with tc.tile_pool(name="persist", bufs=1) as pp, \
             tc.tile_pool(name="pswap", bufs=2) as pswap, \
             tc.tile_pool(name="ppt", bufs=4) as ppt, \
             tc.tile_pool(name="pdib", bufs=2) as pdib, \
             tc.tile_pool(name="p3", bufs=4) as p3, \
             tc.tile_pool(name="ps", bufs=1, space="PSUM") as psp:
            qT = pp.tile([P, NH, S], F16)
            vT = pp.tile([P, S], F16)
            attnT16 = pp.tile([P, NH, QCH], F16)
            kT = pp.tile([P, S], F16)
            vKf = pp.tile([P, NKC, P], F16)      # V as (kpos, kchunk, hd)
            vK8 = pp.tile([P, NKC, P], F8)
            attnT8 = pp.tile([P, NH, S], F8)      # jq>=1 attention out
            wo16 = pp.tile([P, NH, D], F16)
            wo8 = pp.tile([P, NH, D], F8)
            tri_t = pp.tile([P, P], F16)
            mbias_t = pp.tile([P, 4, QCH], F16)
            ones16_t = pp.tile([P, P], F16)
            ones8_t = pp.tile([P, 2, P], F8)
            cos_t = pp.tile([P, S], F16)
            sin_t = pp.tile([P, S], F16)
            eb = pp.tile([P, 1], F32)
            nc.gpsimd.memset(eb[:], -1.0)

            wkt = pp.tile([P, NKD, P], F16)
            wvt = pp.tile([P, NKD, P], F16)
            wqt = pp.tile([P, NKD, QW], F16)
            xf = pp.tile([P, NKD, S], F16)

            # DMA priority order: first-needed first.
            nc.sync.dma_start(tri_t[:], tri_d[:])
            nc.sync.dma_start(wkt[:], wk_v)
            for jr in range(NQC):
                nc.sync.dma_start(
                    xf[:, 0, jr * QCH : (jr + 1) * QCH],
                    xT_v[:, 0, jr * QCH : (jr + 1) * QCH],
                )
            nc.sync.dma_start(wvt[:], wv_v)
            for kd in range(1, NKD):
                nc.sync.dma_start(xf[:, kd, :], xT_v[:, kd, :])
            nc.sync.dma_start(cos_t[:], cos_d[:])
            nc.sync.dma_start(sin_t[:], sin_d[:])
            nc.sync.dma_start(mbias_t[:], mbias_d[:])
            nc.sync.dma_start(ones16_t[:], ones16_d[:])
            nc.sync.dma_start(
                ones8_t[:], ones8_d[:].rearrange("p (a b) -> p a b", a=2)
            )
            nc.sync.dma_start(wqt[:], wq_v)
            nc.sync.dma_start(wo16[:], wo_v)
            nc.scalar.copy(out=wo8[:], in_=wo16[:])

            # PSUM tags: A,B = [128,1024] (2 banks each); C..F = [128,512].
            def psA(name):
                return psp.tile([P, 2 * QCH], F32, tag=name, name=name)

            def psB(name):
                return psp.tile([P, QCH], F32, tag=name, name=name)

            def rope(dst_ap):
                sw = pswap.tile([P, S], F16, tag="swap")
                half = P // 2
                nc.sync.dma_start(sw[:half, :], dst_ap[half:, :])
                nc.sync.dma_start(sw[half:, :], dst_ap[:half, :])
                nc.vector.tensor_tensor(sw[:], sw[:], sin_t[:], OP.mult)
                nc.vector.tensor_tensor(dst_ap, dst_ap, cos_t[:], OP.mult)
                nc.vector.tensor_tensor(dst_ap, dst_ap, sw[:], OP.add)

            # PE warmup while the first DMAs stream in: eb is memset on
            # device (no DMA dependency), so the PE can start ramping within
            # ~2us of kernel start; a few tri_t matmuls then keep it busy
            # until the first x chunk lands.
            wps = psB("C")
            for i in range(110):
                nc.tensor.matmul(
                    wps[0:1, 0:1], eb[:], eb[:], start=True, stop=True
                )
            for i in range(16):
                nc.tensor.matmul(
                    wps[:, 0:P], tri_t[:], tri_t[:], start=True, stop=True
                )

            # ======== Phase 1: QKV projections ==========================
            # K/V stream kd-outer (start on the first x chunk); K uses
            # banks A,B (1024-wide halves), V uses C..F.
            kA, kB = psA("A"), psA("B")
            vps = [psB(t) for t in "CDEF"]
            for kd in range(NKD):
                for jr in range(NQC):
                    dst = (kA, kB)[jr // 2][:, (jr % 2) * QCH : (jr % 2 + 1) * QCH]
                    nc.tensor.matmul(
                        dst,
                        wkt[:, kd, :],
                        xf[:, kd, jr * QCH : (jr + 1) * QCH],
                        start=(kd == 0),
                        stop=(kd == NKD - 1),
                    )
                for jr in range(NQC):
                    nc.tensor.matmul(
                        vps[jr][:],
                        wvt[:, kd, :],
                        xf[:, kd, jr * QCH : (jr + 1) * QCH],
                        start=(kd == 0),
                        stop=(kd == NKD - 1),
                    )
            nc.scalar.copy(out=kT[:, 0 : 2 * QCH], in_=kA[:])
            nc.scalar.copy(out=kT[:, 2 * QCH : S], in_=kB[:])
            rope(kT[:])
            for jr in range(NQC):
                nc.scalar.copy(
                    out=vT[:, jr * QCH : (jr + 1) * QCH], in_=vps[jr][:]
                )
            nc.sync.dma_start_transpose(vKf[:], vT[:])
            nc.scalar.copy(out=vK8[:], in_=vKf[:])

            # Q heads: even heads use banks A,B; odd heads use C..F so the
            # next head's matmuls never WAR-stall on the previous head's
            # PSUM->SBUF copies. The first jq0 attention group is emitted
            # between q1 and q2 so its exps overlap the q2/q3 projections.
            def proj_q(hh):
                if hh % 2 == 0:
                    qab = [psA("A"), psA("B")]
                    qdst = [
                        qab[jr // 2][:, (jr % 2) * QCH : (jr % 2 + 1) * QCH]
                        for jr in range(NQC)
                    ]
                else:
                    qcf = [psB(t) for t in "CDEF"]
                    qdst = [qcf[jr][:] for jr in range(NQC)]
                for kd in range(NKD):
                    for jr in range(NQC):
                        nc.tensor.matmul(
                            qdst[jr],
                            wqt[:, kd, hh * P : (hh + 1) * P],
                            xf[:, kd, jr * QCH : (jr + 1) * QCH],
                            start=(kd == 0),
                            stop=(kd == NKD - 1),
                        )
                if hh % 2 == 0:
                    nc.scalar.copy(out=qT[:, hh, 0 : 2 * QCH], in_=qab[0][:])
                    nc.scalar.copy(out=qT[:, hh, 2 * QCH : S], in_=qab[1][:])
                else:
                    for jr in range(NQC):
                        nc.scalar.copy(
                            out=qT[:, hh, jr * QCH : (jr + 1) * QCH],
                            in_=qcf[jr][:],
                        )
                rope(qT[:, hh, :])

            # ======== Fused attention + output projection ================
            # Two heads co-scheduled per group; sps double-buffers on the
            # global pair index so QK(next) overlaps exp(cur). The second
            # head walks its key pairs in reverse so the two heads' diagonal
            # (masked) pairs never land in the same slot (spreads DVE work).
            pair_it = [0]
            pending_units = []

            def next_ab():
                sps = psA("A" if pair_it[0] % 2 == 0 else "B")
                pair_it[0] += 1
                return sps

            def attn_group(ha, hb, jq):
                fp8 = jq >= 1
                nkc = 4 * (jq + 1)
                npair = nkc // 2
                ops = {ha: psB("C"), hb: psB("D")}
                dps = {ha: psB("E"), hb: psB("F")}
                qs = {
                    h: qT[:, h, jq * QCH : (jq + 1) * QCH] for h in (ha, hb)
                }
                seq = []
                for i in range(npair):
                    seq.append((ha, i))
                    seq.append((hb, npair - 1 - i))
                first = {ha: 0, hb: 1}
                last = {ha: len(seq) - 2, hb: len(seq) - 1}

                def emit_qk(h, ip):
                    sps = next_ab()
                    kc0 = 2 * ip
                    for k2 in range(2):
                        jd = kc0 + k2 - 4 * jq
                        diag = jd >= 0
                        half = sps[:, k2 * QCH : (k2 + 1) * QCH]
                        nc.tensor.matmul(
                            half,
                            kT[:, (kc0 + k2) * P : (kc0 + k2 + 1) * P],
                            qs[h],
                            start=True,
                            stop=not diag,
                        )
                        if diag:
                            ncols = min(QCH, P * (jd + 1))
                            nc.tensor.matmul(
                                half[:, :ncols],
                                tri_t[:],
                                mbias_t[:, jd, :ncols],
                                start=False,
                                stop=True,
                            )
                    return sps

                sps_cur = emit_qk(*seq[0])
                for i, (h, ip) in enumerate(seq):
                    kc0 = 2 * ip
                    if fp8:
                        pT = ppt.tile([P, 2, QCH], F8, tag="pT8")
                    else:
                        pT = ppt.tile([P, 2, QCH], F16, tag="pT16")
                    pflat = pT[:].rearrange("p a b -> p (a b)")
                    nc.scalar.activation(
                        pflat, sps_cur[:], AF.Exp, scale=SCALE, bias=eb[:]
                    )
                    if i + 1 < len(seq):
                        sps_cur = emit_qk(*seq[i + 1])
                    if i % 2 == 1 and pending_units:
                        # fill the exp-latency window with an independent
                        # O-proj unit before the exp-gated PV
                        pending_units.pop(0)()
                    if fp8:
                        nc.tensor.matmul(
                            ops[h][:],
                            vK8[:, kc0 : kc0 + 2, :],
                            pT[:],
                            start=(i == first[h]),
                            stop=(i == last[h]),
                            perf_mode=DR,
                        )
                        nc.tensor.matmul(
                            dps[h][:],
                            ones8_t[:],
                            pT[:],
                            start=(i == first[h]),
                            stop=(i == last[h]),
                            perf_mode=DR,
                        )
                    else:
                        for k2 in range(2):
                            nc.tensor.matmul(
                                ops[h][:],
                                vKf[:, kc0 + k2, :],
                                pT[:, k2, :],
                                start=(i == first[h] and k2 == 0),
                                stop=(i == last[h] and k2 == 1),
                            )
                            nc.tensor.matmul(
                                dps[h][:],
                                ones16_t[:],
                                pT[:, k2, :],
                                start=(i == first[h] and k2 == 0),
                                stop=(i == last[h] and k2 == 1),
                            )
                for h in (ha, hb):
                    dib = pdib.tile([P, QCH], F32, tag="dib")
                    nc.vector.reciprocal_approx_fast(dib[:], dps[h][:])
                    if fp8:
                        dst = attnT8[:, h, jq * QCH : (jq + 1) * QCH]
                    else:
                        dst = attnT16[:, h, :]
                    nc.vector.tensor_tensor(dst, ops[h][:], dib[:], OP.mult)

            def make_oproj_units(jq):
                units = []
                for op_i in range(D // P // 2):
                    def unit(op_i=op_i, jq=jq):
                        ps = next_ab()
                        for i2 in range(2):
                            oc = 2 * op_i + i2
                            half = ps[:, i2 * QCH : (i2 + 1) * QCH]
                            if jq == 0:
                                for a in range(NH):
                                    nc.tensor.matmul(
                                        half,
                                        wo16[:, a, oc * P : (oc + 1) * P],
                                        attnT16[:, a, :],
                                        start=(a == 0),
                                        stop=(a == NH - 1),
                                    )
                            else:
                                for a2 in range(0, NH, 2):
                                    nc.tensor.matmul(
                                        half,
                                        wo8[:, a2 : a2 + 2, oc * P : (oc + 1) * P],
                                        attnT8[
                                            :,
                                            a2 : a2 + 2,
                                            jq * QCH : (jq + 1) * QCH,
                                        ],
                                        start=(a2 == 0),
                                        stop=(a2 == NH - 2),
                                        perf_mode=DR,
                                    )
                        ot = p3.tile([P, 2 * QCH], F16, tag="ot")
                        tail = jq == NQC - 1 and op_i >= 6
                        if tail:
                            # last units: split halves across both engines so
                            # the copy+DMA tail drains in parallel
                            nc.scalar.copy(out=ot[:, :QCH], in_=ps[:, :QCH])
                            nc.vector.tensor_copy(
                                out=ot[:, QCH:], in_=ps[:, QCH:]
                            )
                        elif op_i % 2 == 0:
                            nc.scalar.copy(out=ot[:], in_=ps[:])
                        else:
                            nc.vector.tensor_copy(out=ot[:], in_=ps[:])
                        for i2 in range(2):
                            oc = 2 * op_i + i2
                            nc.sync.dma_start(
                                out_d[
                                    oc * P : (oc + 1) * P,
                                    jq * QCH : (jq + 1) * QCH,
                                ],
                                ot[:, i2 * QCH : (i2 + 1) * QCH],
                            )
                    units.append(unit)
                return units

            for hh in range(NH):
                proj_q(hh)
            for jq in range(NQC):
                attn_group(0, 1, jq)
                attn_group(2, 3, jq)
                pending_units.extend(make_oproj_units(jq))
            while pending_units:
                pending_units.pop(0)()

    nc.finalize()
    return nc


_NC = None


def _get_nc():
    global _NC
    if _NC is None:
        _NC = build_nc()
    return _NC


def make_in_maps(x, wq, wk, wv, wo):
    x = np.asarray(x, dtype=np.float32)
    f16 = np.float16
    in_maps = []
    for c in range(8):
        b, g = c // 4, c % 4
        in_maps.append(
            {
                "xT": np.ascontiguousarray(x[b].T).astype(f16),
                "wq": np.asarray(wq[:, QW * g : QW * (g + 1)], dtype=f16),
                "wk": np.asarray(wk[:, P * g : P * (g + 1)], dtype=f16),
                "wv": np.asarray(wv[:, P * g : P * (g + 1)], dtype=f16),
                "wo": np.asarray(wo[QW * g : QW * (g + 1), :], dtype=f16),
            }
        )
    return in_maps


def kernel(x, wq, wk, wv, wo):
    nc = _get_nc()
    in_maps = make_in_maps(x, wq, wk, wv, wo)
    res = run_bass_kernel_spmd(nc, in_maps, list(range(8)))
    parts = [res.results[c]["outT"].astype(np.float32) for c in range(8)]
    out = np.stack(
        [
            (parts[0] + parts[1] + parts[2] + parts[3]).T,
            (parts[4] + parts[5] + parts[6] + parts[7]).T,
        ]
    ).astype(np.float32)
    return out
